# revision 89
# baseline (speedup 1.0000x reference)
"""Single-head causal attention (B=4, T=2048, C=1024) on 8 trn2 NeuronCores.

Sharding: 8 shards = (batch b in 0..3) x (query interleave h in 0..1).
Query rows are sharded as interleaved 64-row blocks (core h takes global
blocks {2*i+h}), which splits every kv tile's 128-row causal diagonal band
exactly 50/50 across the core pair: kv tile s is valid for local query
cols [64*s, 1024) on BOTH cores. One SPMD instruction stream serves all
cores; all per-core variation is data: gathered x slices, a per-kv-row
score bias, and a single [128,64] mask tile holding this core's half of
the diagonal band.

Host-side weight fusions (both exact):
  scores = (W_q x_q + b_q) . (W_k x_kv + b_k) / sqrt(C)
         = x_q^T M x_kv + c(kv) + g(q),   M = W_q^T W_k / sqrt(C)
    where c_s = b_q . k_s / sqrt(C) enters exp() as a per-kv-partition bias
    (host-computed rank-1 stat: x_kv @ W_k^T b_q + b_q.b_k), and g(q) is
    constant per query column so softmax cancels it -- dropped.
    => the q-projection never runs on device.
  out = softmax(S) V W_p^T + b = (A (W_p W_v) x)/rowsum + (b_p + W_p b_v)
    since softmax rows sum to one => U = W_u x with W_u = W_p W_v replaces
    the V-projection, and the output projection never runs on device.

Device layout per core (S^T formulation -- scores kept as [kv, query] so
att@U needs no transposes):
  zproj: z = M x for all 2048 kv rows into full-width zT[ct][128,2048];
    the first x-half streams against a pair-wise ct-outer matmul order so
    the PE stays ahead of the DMA at kernel start (plus a few 1-row
    const-tile warmup matmuls that start the p-state ramp clock early).
  uproj: U = W_u x into 16 [128,1024] tiles.
  scores: one pass over all 16 kv tiles s against raw x_q on exact column
    ranges [64*s, 1024); exp(score + c_s) on Act -> AT[s] (bf16). The DVE
    accumulates the exp'd tiles into an f32r rs_acc as they appear; two
    f32r ones-matmuls (1 cycle/row) then collapse the 128 kv lanes into
    broadcast row-sums, emitted after the first att@U group so the PE
    never waits on the DVE chain.
  att@U: per output-channel tile, a single PSUM accumulation over all 16
    kv tiles on exact column ranges; the DVE multiplies the fp32 PSUM by
    1/rowsum and adds the folded output bias in-register, and the fp32
    result is the final y^T, DMAd out tile-major.

All matmul operands are bf16 (PSUM accumulates fp32): in the TRN2 cost
model bf16 matmuls run at the same 1 cycle/row as float32r but halve every
DMA transfer and SBUF footprint, letting z/U/x for the full sequence stay
resident with no DRAM spill.
"""

import sys

sys.path.insert(0, "/opt/trn_rl_repo")

import numpy as np

import concourse.bass as bass
import concourse.tile as tile
from concourse import mybir
from concourse.vector_clock import ScopedClock

FP = mybir.dt.float32
FPR = mybir.dt.float32r
BF = mybir.dt.bfloat16
AF = mybir.ActivationFunctionType

P = 128
C = 1024  # embed dim
H = 1024  # query rows per core
T = 2048  # kv length
NT = C // P  # 8 tiles of 128
NKV = T // P  # 16 kv tiles
NEG = -1.0e9

_MAX_WAITS = 1


class _TC(tile.TileContext):
    """TileContext whose tail drain puts its global-clock waits on a nop
    (walrus rejects multi-wait Drain); excess waits are split by
    _split_waits() afterwards."""

    def _drain_and_barrier(self, tick_clock, wait_clock):
        nop_inst = self.nc.sync.nop(nofuse=True, hint="pre_drain_waits")
        wait_clock.add_sem_waits(
            nop_inst.ins, ScopedClock({None: tick_clock.global_clock})
        )
        self.nc.sync.drain()
        self.nc.all_engine_barrier()
        assert self.sems is not None
        popped = self.nc._tile_sem_poison_stack.pop()
        assert popped is self._sem_poison
        self.nc.clear_and_free_semaphores(list(self.sems.allocated().values()))
        self.nc.all_engine_barrier()


def _split_waits(nc, max_waits=_MAX_WAITS):
    """The walrus shipped here rejects instructions carrying more than
    `max_waits` sync waits. Move excess waits onto injected nops placed
    immediately before the instruction on the same engine (identical
    semantics: the engine's sequencer blocks on all of them either way)."""
    import copy

    template = nc.sync.nop(nofuse=True, hint="waitsplit_template").ins
    counter = [0]

    def make_nop(engine, waits):
        nop = copy.deepcopy(template)
        counter[0] += 1
        nop.name = f"I-wsplit-{counter[0]}"
        nop.engine = engine
        nop.sync_info = mybir.SyncInfo(on_wait=list(waits), on_update=[])
        return nop

    f = nc.m.functions[0]
    for bb in f.blocks:
        insts = bb.instructions
        if not any(
            i.sync_info and i.sync_info.on_wait and len(i.sync_info.on_wait) > max_waits
            for i in insts
        ):
            continue
        newlist = []
        for inst in insts:
            si = inst.sync_info
            if si and si.on_wait and len(si.on_wait) > max_waits:
                if inst.name == template.name:
                    newlist.append(inst)
                    continue
                waits = list(si.on_wait)
                del si.on_wait[max_waits:]
                rest = waits[max_waits:]
                while rest:
                    newlist.append(make_nop(inst.engine, rest[:max_waits]))
                    rest = rest[max_waits:]
            newlist.append(inst)
        bb.instructions[:] = newlist


def _chunks(lo, hi, step=512):
    out = []
    while lo < hi:
        w = min(step, hi - lo)
        out.append((lo, lo + w))
        lo += w
    return out


def _build_nc():
    nc = bass.Bass("TRN2", target_bir_lowering=False, debug=False)

    xTq = nc.dram_tensor("xTq", [C, H], BF, kind="ExternalInput").ap()
    xTo = nc.dram_tensor("xTo", [C, H], BF, kind="ExternalInput").ap()
    xTx = nc.dram_tensor("xTx", [C, H], BF, kind="ExternalInput").ap()
    zM = nc.dram_tensor("zM", [C, C], BF, kind="ExternalInput").ap()
    uW = nc.dram_tensor("uW", [2 * P, 8 * 512], BF, kind="ExternalInput").ap()
    cb = nc.dram_tensor("cb", [P, NKV], FP, kind="ExternalInput").ap()
    beff = nc.dram_tensor("beff", [P, NT], FP, kind="ExternalInput").ap()
    ones_in = nc.dram_tensor("ones_in", [P, P], FPR, kind="ExternalInput").ap()
    beffd = nc.dram_tensor("beffd", [P, C], BF, kind="ExternalInput").ap()
    m64_in = nc.dram_tensor("m64_in", [P, 64], FP, kind="ExternalInput").ap()
    # output in (ot-tile, chunk)-major layout; host reassembles
    yT = nc.dram_tensor("yT", [NT * 2 * P, 512], FP, kind="ExternalOutput").ap()

    with _TC(nc) as tc:
        with (
            tc.tile_pool(name="misc", bufs=1) as misc,
            tc.tile_pool(name="wstream", bufs=3) as wsp,
            tc.tile_pool(name="wcolp", bufs=1) as wcp,
            tc.tile_pool(name="kqv", bufs=1) as kqv,
            tc.tile_pool(name="evac", bufs=5) as evac,
            tc.tile_pool(name="psum", bufs=8, space="PSUM") as pp,
        ):
            m64 = misc.tile([P, 64], FP, tag="m64")
            cb_sb = misc.tile([P, NKV], FP, tag="cb")
            beff_sb = misc.tile([P, NT], FP, tag="beff")
            beffd_sb = misc.tile([P, C], BF, tag="beffd")

            # ---- persistent tensors --------------------------------------
            zT = [kqv.tile([P, T], BF, tag=f"zT{i}", name=f"zT{i}") for i in range(NT)]
            U = [kqv.tile([P, C], BF, tag=f"U{i}", name=f"U{i}") for i in range(NKV)]
            AT = [kqv.tile([P, H], BF, tag=f"AT{i}", name=f"AT{i}") for i in range(NKV)]
            ATn = [kqv.tile([P, P], BF, tag=f"ATn{i}", name=f"ATn{i}") for i in range(NKV)]
            rs_acc = kqv.tile([P, H], FPR, tag="rs_acc")
            rs_sb = kqv.tile([P, H], FP, tag="rs_sb")
            ones_r = misc.tile([P, P], FPR, tag="ones_r")

            xho = [kqv.tile([P, H], BF, tag=f"xho{i}", name=f"xho{i}") for i in range(NT)]
            xhx = [kqv.tile([P, H], BF, tag=f"xhx{i}", name=f"xhx{i}") for i in range(NT)]
            xq = [kqv.tile([P, H], BF, tag=f"xq{i}", name=f"xq{i}") for i in range(NT)]
            xhalf = [xho, xhx]

            # =============================================================
            # Projections: z = M x, U = W_u x over all 2048 kv rows
            # =============================================================
            # Warm-up: 1-row matmuls on a framework const tile start the PE
            # p-state ramp clock ~4us before the first real matmul; a few
            # 512-row matmuls on a memset tile then keep the PE from idling
            # long enough (>3us) for the ramp to reset before the first
            # data-dependent matmul issues -- so it runs at full frequency.
            ones1 = nc.const_aps.tensor(1.0, [P, 1], BF)
            dummy_sb = misc.tile([P, 512], BF, tag="dummy")
            nc.gpsimd.memset(dummy_sb[:], 1.0)
            warm_ps = pp.tile([P, 512], FP, tag="ps", name="warm_ps")
            for _ in range(4):
                nc.tensor.matmul(
                    warm_ps[0:1, 0:1], lhsT=ones1, rhs=ones1, start=True, stop=True
                )
            for _ in range(1):
                nc.tensor.matmul(
                    warm_ps[0:1, :], lhsT=ones1, rhs=dummy_sb[:],
                    start=True, stop=True,
                )

            # Interleave the zM-column and x-half-0 loads so both streams
            # arrive just in time for the pair-wise ct-outer start below.
            wz0 = [
                wcp.tile([P, C], BF, tag=f"wz{i}", name=f"wz0_{i}") for i in range(NT)
            ]
            for i, j in ((0, None), (None, 0), (1, None), (None, 1), (None, 2),
                         (2, None), (3, None), (None, 3), (None, 4), (None, 5),
                         (4, None), (5, None), (None, 6), (None, 7),
                         (6, None), (7, None)):
                if i is not None:
                    nc.sync.dma_start(wz0[i][:], zM[i * P : (i + 1) * P, :])
                else:
                    nc.sync.dma_start(xho[j][:], xTo[j * P : (j + 1) * P, :])

            def zproj(half, wcols=None, groups=None):
                # z^T: out tile [zc:128, t-chunk], lhsT = M-col slice.
                # `groups` batches ot-tiles with a ct-outer matmul order so
                # each arriving x tile feeds len(group)*1024 rows of PE work
                # (keeps the PE ahead of the x DMA stream at kernel start).
                if groups is None:
                    groups = [[ot] for ot in range(NT)]
                for group in groups:
                    pss = {}
                    if wcols is None:
                        wcols = {}
                    for ot in group:
                        osl = slice(ot * P, (ot + 1) * P)
                        if ot not in wcols:
                            wcols[ot] = wcp.tile(
                                [P, C], BF, tag=f"wz{ot}", name=f"wz{half}_{ot}"
                            )
                            nc.sync.dma_start(wcols[ot][:], zM[osl, :])
                        for (cs, ce) in _chunks(0, H):
                            pss[ot, cs] = pp.tile(
                                [P, 512], FP, tag="ps", name=f"psz{half}_{ot}_{cs}"
                            )
                    for ct in range(NT):
                        for ot in group:
                            for (cs, ce) in _chunks(0, H):
                                nc.tensor.matmul(
                                    pss[ot, cs][:, : ce - cs],
                                    lhsT=wcols[ot][:, ct * P : (ct + 1) * P],
                                    rhs=xhalf[half][ct][:, cs:ce],
                                    start=(ct == 0),
                                    stop=(ct == NT - 1),
                                )
                    for ot in group:
                        for (cs, ce) in _chunks(0, H):
                            nc.scalar.activation(
                                zT[ot][:, half * H + cs : half * H + ce],
                                pss[ot, cs][:, : ce - cs],
                                AF.Identity,
                            )

            def uproj(half):
                # U: out tile [t:128, o-chunk of 512], lhsT = xh col slice
                for oc in range(2):
                    wvoc = wsp.tile(
                        [P, NT * 512], BF, tag="wvoc", bufs=2, name=f"wu{half}_{oc}"
                    )
                    nc.sync.dma_start(wvoc[:], uW[oc * P : (oc + 1) * P, :])
                    ocs = slice(oc * 512, (oc + 1) * 512)
                    # half 1 runs tt descending so the U tiles att@U consumes
                    # last are DVE-copied first (no stall at the AV boundary)
                    tts = range(NT - 1, -1, -1) if half == 1 else range(NT)
                    for tt in tts:
                        ps = pp.tile([P, 512], FP, tag="ps", name=f"psu{half}_{oc}_{tt}")
                        tsl = slice(tt * P, (tt + 1) * P)
                        for ct in range(NT):
                            nc.tensor.matmul(
                                ps[:],
                                lhsT=xhalf[half][ct][:, tsl],
                                rhs=wvoc[:, ct * 512 : (ct + 1) * 512],
                                start=(ct == 0),
                                stop=(ct == NT - 1),
                            )
                        nc.vector.tensor_copy(U[half * NT + tt][:, ocs], ps[:])

            sc = tc.nc.named_scope("A_z0"); sc.__enter__()
            zproj(0, wcols=dict(enumerate(wz0)),
                  groups=[[0, 1], [2, 3], [4, 5], [6, 7]])
            sc.__exit__(None, None, None)

            # later loads: queue behind the critical zproj weight stream
            for i in range(NT):
                nc.sync.dma_start(xhx[i][:], xTx[i * P : (i + 1) * P, :])
            for i in range(NT):
                nc.sync.dma_start(xq[i][:], xTq[i * P : (i + 1) * P, :])
            nc.sync.dma_start(cb_sb[:], cb[:])
            nc.sync.dma_start(ones_r[:], ones_in[:])
            nc.sync.dma_start(m64[:], m64_in[:])
            nc.sync.dma_start(beff_sb[:], beff[:])
            nc.sync.dma_start(beffd_sb[:], beffd[:])

            sc = tc.nc.named_scope("A_z1"); sc.__enter__()
            zproj(1)
            sc.__exit__(None, None, None)
            sc = tc.nc.named_scope("A_u0"); sc.__enter__()
            uproj(0)
            sc.__exit__(None, None, None)
            sc = tc.nc.named_scope("A_u1"); sc.__enter__()
            uproj(1)
            sc.__exit__(None, None, None)

            # =============================================================
            # Attention: scores -> exp -> rowsums, then att@U (one pass)
            # =============================================================
            # kv tile s is valid for local query cols [64*s, 1024): the
            # 64-row query interleave splits each kv tile's diagonal band
            # 50/50 across the core pair, and one s-independent [128,64]
            # mask tile (the core's half of the band) covers the boundary.
            sc = tc.nc.named_scope("S"); sc.__enter__()
            # row-sums: DVE accumulates the exp'd tiles into rs_acc while
            # scores stream; one pair of f32r ones-matmuls then collapses
            # the 128 kv lanes (and broadcasts) -- 1024 PE rows instead of
            # the 8704 a per-tile ones-matmul rowsum would cost.
            for s in range(NKV):
                lo = 64 * s
                for ci, (cs, ce) in enumerate(_chunks(lo, H)):
                    ps = pp.tile([P, 512], FP, tag="ps", name=f"pss{s}_{cs}")
                    w = ce - cs
                    for ct in range(NT):
                        nc.tensor.matmul(
                            ps[:, :w],
                            lhsT=zT[ct][:, s * P : (s + 1) * P],
                            rhs=xq[ct][:, cs:ce],
                            start=(ct == 0),
                            stop=(ct == NT - 1),
                        )
                    if ci == 0:
                        nc.vector.tensor_add(ps[:, 0:64], ps[:, 0:64], m64[:])
                    nc.scalar.activation(
                        AT[s][:, cs:ce], ps[:, :w], AF.Exp,
                        bias=cb_sb[:, s : s + 1],
                    )
                if s == 0:
                    nc.vector.tensor_copy(rs_acc[:], AT[0][:])
                else:
                    nc.vector.tensor_add(
                        rs_acc[:, lo:H], rs_acc[:, lo:H], AT[s][:, lo:H]
                    )
            def rs_collapse():
                for (cs, ce) in _chunks(0, H):
                    ps = pp.tile([P, 512], FP, tag="ps", name=f"psrs_{cs}")
                    nc.tensor.matmul(
                        ps[:], lhsT=ones_r[:], rhs=rs_acc[:, cs:ce],
                        start=True, stop=True,
                    )
                    nc.vector.reciprocal(rs_sb[:, cs:ce], ps[:])
            sc.__exit__(None, None, None)

            sc = tc.nc.named_scope("AV"); sc.__enter__()

            def av_matmuls(ot, cs, ce):
                osl = slice(ot * P, (ot + 1) * P)
                valid = [s for s in range(NKV) if 64 * s < ce]
                ps = pp.tile([P, 512], FP, tag="ps", name=f"psav{ot}_{cs}")
                for s in valid:
                    lo = max(cs, 64 * s)
                    nc.tensor.matmul(
                        ps[:, lo - cs : ce - cs],
                        lhsT=U[s][:, osl],
                        rhs=AT[s][:, lo:ce],
                        start=(s == valid[0]),
                        stop=(s == valid[-1]),
                    )
                return ps

            def av_out(ot, ps, cs, ce):
                # normalize straight out of PSUM, add folded bias, DMA
                # out; both element ops stay on the DVE (same-engine
                # in-order: no cross-engine sem hop on the tail chain)
                w = ce - cs
                ev = evac.tile([P, 512], FP, tag="evy")
                nc.vector.tensor_mul(ev[:, :w], ps[:, :w], rs_sb[:, cs:ce])
                nc.vector.tensor_scalar_add(
                    ev[:, :w], ev[:, :w], beff_sb[:, ot : ot + 1]
                )
                ci = cs // 512
                nc.sync.dma_start(
                    yT[(ot * 2 + ci) * P : (ot * 2 + ci + 1) * P,
                       cs - ci * 512 : ce - ci * 512],
                    ev[:, :w],
                )

            # ot = 0: both matmul groups precede the row-sum lane collapse
            # so the PE's wait on the DVE exp-accumulation chain is hidden
            # under ~3.5us of att@U work
            pss = [av_matmuls(0, cs, ce) for (cs, ce) in ((0, 512), (512, 1024))]
            rs_collapse()
            # pre-normalize the last 128 query cols of every AT tile (DVE
            # idle time) so the final output chunk needs no post-PSUM
            # normalize on its critical chain
            for s in range(NKV):
                nc.vector.tensor_mul(
                    ATn[s][:], AT[s][:, H - P : H], rs_sb[:, H - P : H]
                )
            for ps, (cs, ce) in zip(pss, ((0, 512), (512, 1024))):
                av_out(0, ps, cs, ce)

            for ot in range(1, NT - 2):
                for (cs, ce) in ((0, 512), (512, 1024)):
                    av_out(ot, av_matmuls(ot, cs, ce), cs, ce)

            def av_out_direct(cs, ce):
                # pre-normalized weights + bias folded into the PSUM via a
                # rank-1 ones-matmul -> one copy -> DMA (no DVE math on the
                # critical chain); only valid for cs >= 896 (ATn coverage)
                osl = slice((NT - 1) * P, NT * P)
                w = ce - cs
                ps = pp.tile([P, 512], FP, tag="ps", name=f"psavd_{cs}")
                valid = [s for s in range(NKV) if 64 * s < ce]
                for s in valid:
                    lo = max(cs, 64 * s)
                    nc.tensor.matmul(
                        ps[:, lo - cs : w],
                        lhsT=U[s][:, osl],
                        rhs=ATn[s][:, lo - 896 : ce - 896],
                        start=(s == valid[0]),
                        stop=False,
                    )
                nc.tensor.matmul(
                    ps[:, 0:w],
                    lhsT=beffd_sb[:, osl],
                    rhs=dummy_sb[:, 0:w],
                    start=False,
                    stop=True,
                )
                ev = evac.tile([P, 512], FP, tag="evy")
                nc.vector.tensor_copy(ev[:, 0:w], ps[:, 0:w])
                nc.sync.dma_start(
                    yT[((NT - 1) * 2 + 1) * P : ((NT - 1) * 2 + 2) * P,
                       cs - 512 : ce - 512],
                    ev[:, 0:w],
                )

            for (cs, ce) in ((0, 512), (512, 1024)):
                av_out(NT - 2, av_matmuls(NT - 2, cs, ce), cs, ce)
            # ot 7's non-final chunks first, so their output DMAs mostly
            # clear the HWDGE before the final chunk's critical chain
            for (cs, ce) in ((768, 896), (0, 512), (512, 768)):
                av_out(NT - 1, av_matmuls(NT - 1, cs, ce), cs, ce)
            av_out_direct(896, 1024)
            sc.__exit__(None, None, None)

    _split_waits(nc)
    return nc


_NC_CACHE = None


def _get_nc():
    global _NC_CACHE
    if _NC_CACHE is None:
        _NC_CACHE = _build_nc()
    return _NC_CACHE


def make_in_maps(x, w_qkv, b_qkv, w_proj, b_proj):
    """Host-side prep: weight fusion + shard + transpose + bf16 packing."""
    import ml_dtypes

    BFNP = ml_dtypes.bfloat16
    x = np.asarray(x, dtype=np.float32)
    w_qkv = np.asarray(w_qkv, dtype=np.float32)
    b_qkv = np.asarray(b_qkv, dtype=np.float32)
    w_proj = np.asarray(w_proj, dtype=np.float32)
    b_proj = np.asarray(b_proj, dtype=np.float32)

    s = 1.0 / np.sqrt(np.float32(C))
    Wq = w_qkv[0:C]
    Wk = w_qkv[C : 2 * C]
    Wv = w_qkv[2 * C : 3 * C]
    bqv = b_qkv[0:C]
    bkv = b_qkv[C : 2 * C]
    bvv = b_qkv[2 * C : 3 * C]

    M = (Wq.T @ Wk) * s           # scores main term: x_q^T M x_kv
    Wu = w_proj @ Wv              # fused value/output projection
    beff = b_proj + w_proj @ bvv  # folded output bias
    wc = (Wk.T @ bqv) * s         # c_s = x_s . wc + cconst
    cconst = float(bqv @ bkv) * s

    def pack_cols(w, bw=P):
        # [ot*bw + p(in-part), ct*P + o(out-within)] = w[ot*bw + o, ct*P + p]
        n_o = C // bw
        w4 = w.reshape(n_o, bw, NT, P).transpose(0, 3, 2, 1)
        return np.ascontiguousarray(w4).reshape(n_o * P, NT * bw).astype(BFNP)

    zM = pack_cols(M)
    uW = pack_cols(Wu, bw=512)
    beff_t = np.ascontiguousarray(beff.reshape(NT, P).T)

    # S^T layout: partition = kv index j (0..127 within a kv tile), free =
    # the first valid 64 local query cols; the core sees global query rows
    # 64*h + i2 of the tile's diagonal band: visible iff 64*h + i2 >= j
    jj = np.arange(P)[:, None]
    ii = np.arange(64)[None, :]
    shared = dict(
        zM=zM, uW=uW, beff=beff_t,
        ones_in=np.ones((P, P), dtype=np.float32),
        # bias / 128 broadcast down the contraction partitions: a rank-1
        # ones-matmul reconstitutes beff inside the final output PSUM
        beffd=np.ascontiguousarray(
            np.broadcast_to((beff / P)[None, :], (P, C))
        ).astype(BFNP),
    )
    in_maps = []
    for core in range(8):
        b, h = core // 2, core % 2
        m64 = np.where(64 * h + ii >= jj, 0.0, NEG).astype(np.float32)
        xb = x[b]  # [T, C]
        # per-kv-row score bias c_s, laid out [128, 16] kv-tile-major
        c = (xb @ wc + cconst).astype(np.float32)  # [T]
        cb = np.ascontiguousarray(c.reshape(NKV, P).T)
        # query rows: interleaved 64-blocks g = 2*i + h
        qrows = np.concatenate(
            [xb[(2 * i + h) * 64 : (2 * i + h + 1) * 64] for i in range(H // 64)],
            axis=0,
        )
        xTo_b = np.ascontiguousarray(xb[0:H].T).astype(BFNP)
        in_maps.append(
            dict(
                shared,
                xTq=np.ascontiguousarray(qrows.T).astype(BFNP),
                xTo=xTo_b,
                xTx=np.ascontiguousarray(xb[H : 2 * H].T).astype(BFNP),
                cb=cb,
                m64_in=m64,
            )
        )
    return in_maps


def assemble_output(results):
    B = 4
    y = np.empty((B, T, C), dtype=np.float32)
    for core in range(8):
        b, h = core // 2, core % 2
        # yT layout [ot, ci, p, 512] -> rows are local query cols
        yt = results[core]["yT"].reshape(NT, 2, P, 512)
        blk = yt.transpose(1, 3, 0, 2).reshape(H, C)  # [local q, C]
        blk16 = blk.reshape(H // 64, 64, C)
        for i in range(H // 64):
            g = 2 * i + h
            y[b, g * 64 : (g + 1) * 64, :] = blk16[i]
    return y


def kernel(x, w_qkv, b_qkv, w_proj, b_proj):
    from concourse.bass_utils import run_bass_kernel_spmd

    nc = _get_nc()
    in_maps = make_in_maps(x, w_qkv, b_qkv, w_proj, b_proj)
    res = run_bass_kernel_spmd(nc, in_maps, list(range(8)))
    return assemble_output(res.results)


# revision 91
# speedup vs baseline: 1.1822x; 1.1822x over previous
"""Single-head causal attention (B=4, T=2048, C=1024) on 8 trn2 NeuronCores.

Sharding: 8 shards = (batch b in 0..3) x (query interleave h in 0..1).
Query rows are sharded as interleaved 64-row blocks (core h takes global
blocks {2*i+h}), which splits every kv tile's 128-row causal diagonal band
exactly 50/50 across the core pair: kv tile s is valid for local query
cols [64*s, 1024) on BOTH cores. One SPMD instruction stream serves all
cores; all per-core variation is data: gathered x slices, a per-kv-row
score bias, and a single [128,64] mask tile holding this core's half of
the diagonal band.

Host-side weight fusions (both exact):
  scores = (W_q x_q + b_q) . (W_k x_kv + b_k) / sqrt(C)
         = x_q^T M x_kv + c(kv) + g(q),   M = W_q^T W_k / sqrt(C)
    where c_s = b_q . k_s / sqrt(C) enters exp() as a per-kv-partition bias
    (host-computed rank-1 stat: x_kv @ W_k^T b_q + b_q.b_k), and g(q) is
    constant per query column so softmax cancels it -- dropped.
    => the q-projection never runs on device.
  out = softmax(S) V W_p^T + b = (A (W_p W_v) x)/rowsum + (b_p + W_p b_v)
    since softmax rows sum to one => U = W_u x with W_u = W_p W_v replaces
    the V-projection, and the output projection never runs on device.

Device layout per core (S^T formulation -- scores kept as [kv, query] so
att@U needs no transposes):
  zproj: z = M x for all 2048 kv rows into full-width zT[ct][128,2048];
    the first x-half streams against a pair-wise ct-outer matmul order so
    the PE stays ahead of the DMA at kernel start (plus a few 1-row
    const-tile warmup matmuls that start the p-state ramp clock early).
  uproj: U = W_u x into 16 [128,1024] tiles.
  scores: one pass over all 16 kv tiles s against raw x_q on exact column
    ranges [64*s, 1024); exp(score + c_s) on Act -> AT[s] (bf16). The DVE
    accumulates the exp'd tiles into an f32r rs_acc as they appear; two
    f32r ones-matmuls (1 cycle/row) then collapse the 128 kv lanes into
    broadcast row-sums, emitted after the first att@U group so the PE
    never waits on the DVE chain.
  att@U: per output-channel tile, a single PSUM accumulation over all 16
    kv tiles on exact column ranges; the DVE multiplies the fp32 PSUM by
    1/rowsum and adds the folded output bias in-register, and the fp32
    result is the final y^T, DMAd out tile-major.

All matmul operands are bf16 (PSUM accumulates fp32): in the TRN2 cost
model bf16 matmuls run at the same 1 cycle/row as float32r but halve every
DMA transfer and SBUF footprint, letting z/U/x for the full sequence stay
resident with no DRAM spill.
"""

import sys

sys.path.insert(0, "/opt/trn_rl_repo")

import numpy as np

import concourse.bass as bass
import concourse.tile as tile
from concourse import mybir
from concourse.vector_clock import ScopedClock

FP = mybir.dt.float32
FPR = mybir.dt.float32r
BF = mybir.dt.bfloat16
AF = mybir.ActivationFunctionType

P = 128
C = 1024  # embed dim
H = 1024  # query rows per core
T = 2048  # kv length
NT = C // P  # 8 tiles of 128
NKV = T // P  # 16 kv tiles
NEG = -1.0e9

_MAX_WAITS = 1


class _TC(tile.TileContext):
    """TileContext whose tail drain puts its global-clock waits on a nop
    (walrus rejects multi-wait Drain); excess waits are split by
    _split_waits() afterwards."""

    def _drain_and_barrier(self, tick_clock, wait_clock):
        nop_inst = self.nc.sync.nop(nofuse=True, hint="pre_drain_waits")
        wait_clock.add_sem_waits(
            nop_inst.ins, ScopedClock({None: tick_clock.global_clock})
        )
        self.nc.sync.drain()
        self.nc.all_engine_barrier()
        assert self.sems is not None
        popped = self.nc._tile_sem_poison_stack.pop()
        assert popped is self._sem_poison
        self.nc.clear_and_free_semaphores(list(self.sems.allocated().values()))
        self.nc.all_engine_barrier()


def _split_waits(nc, max_waits=_MAX_WAITS):
    """The walrus shipped here rejects instructions carrying more than
    `max_waits` sync waits. Move excess waits onto injected nops placed
    immediately before the instruction on the same engine (identical
    semantics: the engine's sequencer blocks on all of them either way)."""
    import copy

    template = nc.sync.nop(nofuse=True, hint="waitsplit_template").ins
    counter = [0]

    def make_nop(engine, waits):
        nop = copy.deepcopy(template)
        counter[0] += 1
        nop.name = f"I-wsplit-{counter[0]}"
        nop.engine = engine
        nop.sync_info = mybir.SyncInfo(on_wait=list(waits), on_update=[])
        return nop

    f = nc.m.functions[0]
    for bb in f.blocks:
        insts = bb.instructions
        if not any(
            i.sync_info and i.sync_info.on_wait and len(i.sync_info.on_wait) > max_waits
            for i in insts
        ):
            continue
        newlist = []
        for inst in insts:
            si = inst.sync_info
            if si and si.on_wait and len(si.on_wait) > max_waits:
                if inst.name == template.name:
                    newlist.append(inst)
                    continue
                waits = list(si.on_wait)
                del si.on_wait[max_waits:]
                rest = waits[max_waits:]
                while rest:
                    newlist.append(make_nop(inst.engine, rest[:max_waits]))
                    rest = rest[max_waits:]
            newlist.append(inst)
        bb.instructions[:] = newlist


def _chunks(lo, hi, step=512):
    out = []
    while lo < hi:
        w = min(step, hi - lo)
        out.append((lo, lo + w))
        lo += w
    return out


def _build_nc():
    nc = bass.Bass("TRN2", target_bir_lowering=False, debug=False)

    xTq = nc.dram_tensor("xTq", [C, H], BF, kind="ExternalInput").ap()
    xTo = nc.dram_tensor("xTo", [C, H], BF, kind="ExternalInput").ap()
    xTx = nc.dram_tensor("xTx", [C, H], BF, kind="ExternalInput").ap()
    zM = nc.dram_tensor("zM", [C, C], BF, kind="ExternalInput").ap()
    uW = nc.dram_tensor("uW", [2 * P, 8 * 512], BF, kind="ExternalInput").ap()
    cb = nc.dram_tensor("cb", [P, NKV], FP, kind="ExternalInput").ap()
    beff = nc.dram_tensor("beff", [P, NT], FP, kind="ExternalInput").ap()
    ones_in = nc.dram_tensor("ones_in", [P, P], FPR, kind="ExternalInput").ap()
    beffd = nc.dram_tensor("beffd", [P, C], BF, kind="ExternalInput").ap()
    m64_in = nc.dram_tensor("m64_in", [P, 64], FP, kind="ExternalInput").ap()
    # output in (ot-tile, chunk)-major layout; host reassembles
    yT = nc.dram_tensor("yT", [NT * 2 * P, 512], FP, kind="ExternalOutput").ap()

    with _TC(nc) as tc:
        with (
            tc.tile_pool(name="misc", bufs=1) as misc,
            tc.tile_pool(name="wstream", bufs=3) as wsp,
            tc.tile_pool(name="wcolp", bufs=1) as wcp,
            tc.tile_pool(name="kqv", bufs=1) as kqv,
            tc.tile_pool(name="evac", bufs=5) as evac,
            tc.tile_pool(name="psum", bufs=8, space="PSUM") as pp,
        ):
            m64 = misc.tile([P, 64], FP, tag="m64")
            cb_sb = misc.tile([P, NKV], FP, tag="cb")
            beff_sb = misc.tile([P, NT], FP, tag="beff")
            beffd_sb = misc.tile([P, C], BF, tag="beffd")

            # ---- persistent tensors --------------------------------------
            YT = [kqv.tile([P, H], BF, tag=f"YT{i}", name=f"YT{i}") for i in range(NT)]
            U = [kqv.tile([P, C], BF, tag=f"U{i}", name=f"U{i}") for i in range(NKV)]
            AT = [kqv.tile([P, H], BF, tag=f"AT{i}", name=f"AT{i}") for i in range(NKV)]
            ATn = [kqv.tile([P, P], BF, tag=f"ATn{i}", name=f"ATn{i}") for i in range(NKV)]
            rs_acc = kqv.tile([P, H], FPR, tag="rs_acc")
            rs_sb = kqv.tile([P, H], FP, tag="rs_sb")
            ones_r = misc.tile([P, P], FPR, tag="ones_r")

            xho = [kqv.tile([P, H], BF, tag=f"xho{i}", name=f"xho{i}") for i in range(NT)]
            xhx = [kqv.tile([P, H], BF, tag=f"xhx{i}", name=f"xhx{i}") for i in range(NT)]
            xq = [kqv.tile([P, H], BF, tag=f"xq{i}", name=f"xq{i}") for i in range(NT)]
            xhalf = [xho, xhx]

            # =============================================================
            # Projections: z = M x, U = W_u x over all 2048 kv rows
            # =============================================================
            # Warm-up: 1-row matmuls on a framework const tile start the PE
            # p-state ramp clock ~4us before the first real matmul; a few
            # 512-row matmuls on a memset tile then keep the PE from idling
            # long enough (>3us) for the ramp to reset before the first
            # data-dependent matmul issues -- so it runs at full frequency.
            ones1 = nc.const_aps.tensor(1.0, [P, 1], BF)
            dummy_sb = misc.tile([P, 512], BF, tag="dummy")
            nc.gpsimd.memset(dummy_sb[:], 1.0)
            warm_ps = pp.tile([P, 512], FP, tag="ps", name="warm_ps")
            for _ in range(4):
                nc.tensor.matmul(
                    warm_ps[0:1, 0:1], lhsT=ones1, rhs=ones1, start=True, stop=True
                )
            for _ in range(1):
                nc.tensor.matmul(
                    warm_ps[0:1, :], lhsT=ones1, rhs=dummy_sb[:],
                    start=True, stop=True,
                )

            # Interleave the zM-column and x-half-0 loads so both streams
            # arrive just in time for the pair-wise ct-outer start below.
            wz0 = [
                wcp.tile([P, C], BF, tag=f"wz{i}", name=f"wz0_{i}") for i in range(NT)
            ]
            for i, j in ((0, None), (None, 0), (1, None), (None, 1), (None, 2),
                         (2, None), (3, None), (None, 3), (None, 4), (None, 5),
                         (4, None), (5, None), (None, 6), (None, 7),
                         (6, None), (7, None)):
                if i is not None:
                    nc.sync.dma_start(wz0[i][:], zM[i * P : (i + 1) * P, :])
                else:
                    nc.sync.dma_start(xq[j][:], xTq[j * P : (j + 1) * P, :])
            for i in range(NT):
                nc.sync.dma_start(xho[i][:], xTo[i * P : (i + 1) * P, :])

            def yproj(wcols=None, groups=None):
                # z^T: out tile [zc:128, t-chunk], lhsT = M-col slice.
                # `groups` batches ot-tiles with a ct-outer matmul order so
                # each arriving x tile feeds len(group)*1024 rows of PE work
                # (keeps the PE ahead of the x DMA stream at kernel start).
                if groups is None:
                    groups = [[ot] for ot in range(NT)]
                for group in groups:
                    pss = {}
                    if wcols is None:
                        wcols = {}
                    for ot in group:
                        osl = slice(ot * P, (ot + 1) * P)
                        if ot not in wcols:
                            wcols[ot] = wcp.tile(
                                [P, C], BF, tag=f"wz{ot}", name=f"wy_{ot}"
                            )
                            nc.sync.dma_start(wcols[ot][:], zM[osl, :])
                        for (cs, ce) in _chunks(0, H):
                            pss[ot, cs] = pp.tile(
                                [P, 512], FP, tag="ps", name=f"psy_{ot}_{cs}"
                            )
                    for ct in range(NT):
                        for ot in group:
                            for (cs, ce) in _chunks(0, H):
                                nc.tensor.matmul(
                                    pss[ot, cs][:, : ce - cs],
                                    lhsT=wcols[ot][:, ct * P : (ct + 1) * P],
                                    rhs=xq[ct][:, cs:ce],
                                    start=(ct == 0),
                                    stop=(ct == NT - 1),
                                )
                    for ot in group:
                        for (cs, ce) in _chunks(0, H):
                            nc.scalar.activation(
                                YT[ot][:, cs:ce],
                                pss[ot, cs][:, : ce - cs],
                                AF.Identity,
                            )

            def uproj(half):
                # U: out tile [t:128, o-chunk of 512], lhsT = xh col slice
                for oc in range(2):
                    wvoc = wsp.tile(
                        [P, NT * 512], BF, tag="wvoc", bufs=2, name=f"wu{half}_{oc}"
                    )
                    nc.sync.dma_start(wvoc[:], uW[oc * P : (oc + 1) * P, :])
                    ocs = slice(oc * 512, (oc + 1) * 512)
                    # half 1 runs tt descending so the U tiles att@U consumes
                    # last are DVE-copied first (no stall at the AV boundary)
                    tts = range(NT - 1, -1, -1) if half == 1 else range(NT)
                    for tt in tts:
                        ps = pp.tile([P, 512], FP, tag="ps", name=f"psu{half}_{oc}_{tt}")
                        tsl = slice(tt * P, (tt + 1) * P)
                        for ct in range(NT):
                            nc.tensor.matmul(
                                ps[:],
                                lhsT=xhalf[half][ct][:, tsl],
                                rhs=wvoc[:, ct * 512 : (ct + 1) * 512],
                                start=(ct == 0),
                                stop=(ct == NT - 1),
                            )
                        nc.vector.tensor_copy(U[half * NT + tt][:, ocs], ps[:])

            sc = tc.nc.named_scope("A_y"); sc.__enter__()
            yproj(wcols=dict(enumerate(wz0)),
                  groups=[[0, 1], [2, 3], [4, 5], [6, 7]])
            sc.__exit__(None, None, None)

            # later loads: queue behind the critical yproj weight stream
            for i in range(NT):
                nc.sync.dma_start(xhx[i][:], xTx[i * P : (i + 1) * P, :])
            nc.sync.dma_start(cb_sb[:], cb[:])
            nc.sync.dma_start(ones_r[:], ones_in[:])
            nc.sync.dma_start(m64[:], m64_in[:])
            nc.sync.dma_start(beff_sb[:], beff[:])
            nc.sync.dma_start(beffd_sb[:], beffd[:])

            sc = tc.nc.named_scope("A_u0"); sc.__enter__()
            uproj(0)
            sc.__exit__(None, None, None)
            sc = tc.nc.named_scope("A_u1"); sc.__enter__()
            uproj(1)
            sc.__exit__(None, None, None)

            # =============================================================
            # Attention: scores -> exp -> rowsums, then att@U (one pass)
            # =============================================================
            # kv tile s is valid for local query cols [64*s, 1024): the
            # 64-row query interleave splits each kv tile's diagonal band
            # 50/50 across the core pair, and one s-independent [128,64]
            # mask tile (the core's half of the band) covers the boundary.
            sc = tc.nc.named_scope("S"); sc.__enter__()
            # row-sums: DVE accumulates the exp'd tiles into rs_acc while
            # scores stream; one pair of f32r ones-matmuls then collapses
            # the 128 kv lanes (and broadcasts) -- 1024 PE rows instead of
            # the 8704 a per-tile ones-matmul rowsum would cost.
            for s in range(NKV):
                lo = 64 * s
                for ci, (cs, ce) in enumerate(_chunks(lo, H)):
                    ps = pp.tile([P, 512], FP, tag="ps", name=f"pss{s}_{cs}")
                    w = ce - cs
                    for ct in range(NT):
                        nc.tensor.matmul(
                            ps[:, :w],
                            lhsT=xhalf[s // NT][ct][:, (s % NT) * P : (s % NT + 1) * P],
                            rhs=YT[ct][:, cs:ce],
                            start=(ct == 0),
                            stop=(ct == NT - 1),
                        )
                    if ci == 0:
                        nc.vector.tensor_add(ps[:, 0:64], ps[:, 0:64], m64[:])
                    nc.scalar.activation(
                        AT[s][:, cs:ce], ps[:, :w], AF.Exp,
                        bias=cb_sb[:, s : s + 1],
                    )
                if s == 0:
                    nc.vector.tensor_copy(rs_acc[:], AT[0][:])
                else:
                    nc.vector.tensor_add(
                        rs_acc[:, lo:H], rs_acc[:, lo:H], AT[s][:, lo:H]
                    )
            def rs_collapse():
                for (cs, ce) in _chunks(0, H):
                    ps = pp.tile([P, 512], FP, tag="ps", name=f"psrs_{cs}")
                    nc.tensor.matmul(
                        ps[:], lhsT=ones_r[:], rhs=rs_acc[:, cs:ce],
                        start=True, stop=True,
                    )
                    nc.vector.reciprocal(rs_sb[:, cs:ce], ps[:])
            sc.__exit__(None, None, None)

            sc = tc.nc.named_scope("AV"); sc.__enter__()

            def av_matmuls(ot, cs, ce):
                osl = slice(ot * P, (ot + 1) * P)
                valid = [s for s in range(NKV) if 64 * s < ce]
                ps = pp.tile([P, 512], FP, tag="ps", name=f"psav{ot}_{cs}")
                for s in valid:
                    lo = max(cs, 64 * s)
                    nc.tensor.matmul(
                        ps[:, lo - cs : ce - cs],
                        lhsT=U[s][:, osl],
                        rhs=AT[s][:, lo:ce],
                        start=(s == valid[0]),
                        stop=(s == valid[-1]),
                    )
                return ps

            def av_out(ot, ps, cs, ce):
                # normalize straight out of PSUM, add folded bias, DMA
                # out; both element ops stay on the DVE (same-engine
                # in-order: no cross-engine sem hop on the tail chain)
                w = ce - cs
                ev = evac.tile([P, 512], FP, tag="evy")
                nc.vector.tensor_mul(ev[:, :w], ps[:, :w], rs_sb[:, cs:ce])
                nc.vector.tensor_scalar_add(
                    ev[:, :w], ev[:, :w], beff_sb[:, ot : ot + 1]
                )
                ci = cs // 512
                nc.sync.dma_start(
                    yT[(ot * 2 + ci) * P : (ot * 2 + ci + 1) * P,
                       cs - ci * 512 : ce - ci * 512],
                    ev[:, :w],
                )

            # ot = 0: both matmul groups precede the row-sum lane collapse
            # so the PE's wait on the DVE exp-accumulation chain is hidden
            # under ~3.5us of att@U work
            pss = [av_matmuls(0, cs, ce) for (cs, ce) in ((0, 512), (512, 1024))]
            rs_collapse()
            # pre-normalize the last 128 query cols of every AT tile (DVE
            # idle time) so the final output chunk needs no post-PSUM
            # normalize on its critical chain
            for s in range(NKV):
                nc.vector.tensor_mul(
                    ATn[s][:], AT[s][:, H - P : H], rs_sb[:, H - P : H]
                )
            for ps, (cs, ce) in zip(pss, ((0, 512), (512, 1024))):
                av_out(0, ps, cs, ce)

            for ot in range(1, NT - 2):
                for (cs, ce) in ((0, 512), (512, 1024)):
                    av_out(ot, av_matmuls(ot, cs, ce), cs, ce)

            def av_out_direct(cs, ce):
                # pre-normalized weights + bias folded into the PSUM via a
                # rank-1 ones-matmul -> one copy -> DMA (no DVE math on the
                # critical chain); only valid for cs >= 896 (ATn coverage)
                osl = slice((NT - 1) * P, NT * P)
                w = ce - cs
                ps = pp.tile([P, 512], FP, tag="ps", name=f"psavd_{cs}")
                valid = [s for s in range(NKV) if 64 * s < ce]
                for s in valid:
                    lo = max(cs, 64 * s)
                    nc.tensor.matmul(
                        ps[:, lo - cs : w],
                        lhsT=U[s][:, osl],
                        rhs=ATn[s][:, lo - 896 : ce - 896],
                        start=(s == valid[0]),
                        stop=False,
                    )
                nc.tensor.matmul(
                    ps[:, 0:w],
                    lhsT=beffd_sb[:, osl],
                    rhs=dummy_sb[:, 0:w],
                    start=False,
                    stop=True,
                )
                ev = evac.tile([P, 512], FP, tag="evy")
                nc.vector.tensor_copy(ev[:, 0:w], ps[:, 0:w])
                nc.sync.dma_start(
                    yT[((NT - 1) * 2 + 1) * P : ((NT - 1) * 2 + 2) * P,
                       cs - 512 : ce - 512],
                    ev[:, 0:w],
                )

            for (cs, ce) in ((0, 512), (512, 1024)):
                av_out(NT - 2, av_matmuls(NT - 2, cs, ce), cs, ce)
            # ot 7's non-final chunks first, so their output DMAs mostly
            # clear the HWDGE before the final chunk's critical chain
            for (cs, ce) in ((768, 896), (0, 512), (512, 768)):
                av_out(NT - 1, av_matmuls(NT - 1, cs, ce), cs, ce)
            av_out_direct(896, 1024)
            sc.__exit__(None, None, None)

    _split_waits(nc)
    return nc


_NC_CACHE = None


def _get_nc():
    global _NC_CACHE
    if _NC_CACHE is None:
        _NC_CACHE = _build_nc()
    return _NC_CACHE


def make_in_maps(x, w_qkv, b_qkv, w_proj, b_proj):
    """Host-side prep: weight fusion + shard + transpose + bf16 packing."""
    import ml_dtypes

    BFNP = ml_dtypes.bfloat16
    x = np.asarray(x, dtype=np.float32)
    w_qkv = np.asarray(w_qkv, dtype=np.float32)
    b_qkv = np.asarray(b_qkv, dtype=np.float32)
    w_proj = np.asarray(w_proj, dtype=np.float32)
    b_proj = np.asarray(b_proj, dtype=np.float32)

    s = 1.0 / np.sqrt(np.float32(C))
    Wq = w_qkv[0:C]
    Wk = w_qkv[C : 2 * C]
    Wv = w_qkv[2 * C : 3 * C]
    bqv = b_qkv[0:C]
    bkv = b_qkv[C : 2 * C]
    bvv = b_qkv[2 * C : 3 * C]

    M = (Wq.T @ Wk) * s           # scores main term: x_q^T M x_kv
    Wu = w_proj @ Wv              # fused value/output projection
    beff = b_proj + w_proj @ bvv  # folded output bias
    wc = (Wk.T @ bqv) * s         # c_s = x_s . wc + cconst
    cconst = float(bqv @ bkv) * s

    def pack_cols(w, bw=P):
        # [ot*bw + p(in-part), ct*P + o(out-within)] = w[ot*bw + o, ct*P + p]
        n_o = C // bw
        w4 = w.reshape(n_o, bw, NT, P).transpose(0, 3, 2, 1)
        return np.ascontiguousarray(w4).reshape(n_o * P, NT * bw).astype(BFNP)

    zM = pack_cols(np.ascontiguousarray(M.T))
    uW = pack_cols(Wu, bw=512)
    beff_t = np.ascontiguousarray(beff.reshape(NT, P).T)

    # S^T layout: partition = kv index j (0..127 within a kv tile), free =
    # the first valid 64 local query cols; the core sees global query rows
    # 64*h + i2 of the tile's diagonal band: visible iff 64*h + i2 >= j
    jj = np.arange(P)[:, None]
    ii = np.arange(64)[None, :]
    shared = dict(
        zM=zM, uW=uW, beff=beff_t,
        ones_in=np.ones((P, P), dtype=np.float32),
        # bias / 128 broadcast down the contraction partitions: a rank-1
        # ones-matmul reconstitutes beff inside the final output PSUM
        beffd=np.ascontiguousarray(
            np.broadcast_to((beff / P)[None, :], (P, C))
        ).astype(BFNP),
    )
    in_maps = []
    for core in range(8):
        b, h = core // 2, core % 2
        m64 = np.where(64 * h + ii >= jj, 0.0, NEG).astype(np.float32)
        xb = x[b]  # [T, C]
        # per-kv-row score bias c_s, laid out [128, 16] kv-tile-major
        c = (xb @ wc + cconst).astype(np.float32)  # [T]
        cb = np.ascontiguousarray(c.reshape(NKV, P).T)
        # query rows: interleaved 64-blocks g = 2*i + h
        qrows = np.concatenate(
            [xb[(2 * i + h) * 64 : (2 * i + h + 1) * 64] for i in range(H // 64)],
            axis=0,
        )
        xTo_b = np.ascontiguousarray(xb[0:H].T).astype(BFNP)
        in_maps.append(
            dict(
                shared,
                xTq=np.ascontiguousarray(qrows.T).astype(BFNP),
                xTo=xTo_b,
                xTx=np.ascontiguousarray(xb[H : 2 * H].T).astype(BFNP),
                cb=cb,
                m64_in=m64,
            )
        )
    return in_maps


def assemble_output(results):
    B = 4
    y = np.empty((B, T, C), dtype=np.float32)
    for core in range(8):
        b, h = core // 2, core % 2
        # yT layout [ot, ci, p, 512] -> rows are local query cols
        yt = results[core]["yT"].reshape(NT, 2, P, 512)
        blk = yt.transpose(1, 3, 0, 2).reshape(H, C)  # [local q, C]
        blk16 = blk.reshape(H // 64, 64, C)
        for i in range(H // 64):
            g = 2 * i + h
            y[b, g * 64 : (g + 1) * 64, :] = blk16[i]
    return y


def kernel(x, w_qkv, b_qkv, w_proj, b_proj):
    from concourse.bass_utils import run_bass_kernel_spmd

    nc = _get_nc()
    in_maps = make_in_maps(x, w_qkv, b_qkv, w_proj, b_proj)
    res = run_bass_kernel_spmd(nc, in_maps, list(range(8)))
    return assemble_output(res.results)


# revision 94
# speedup vs baseline: 1.4422x; 1.2199x over previous
"""Single-head causal attention (B=4, T=2048, C=1024) on 8 trn2 NeuronCores.

Sharding: 8 shards = (batch b in 0..3) x (query interleave h in 0..1).
Query rows are sharded as interleaved 64-row blocks (core h takes global
blocks {2*i+h}), which splits every kv tile's 128-row causal diagonal band
exactly 50/50 across the core pair: kv tile s is valid for local query
cols [64*s, 1024) on BOTH cores. One SPMD instruction stream serves all
cores; all per-core variation is data: gathered x slices, a per-kv-row
score bias, and a single [128,64] mask tile holding this core's half of
the diagonal band.

Host-side weight fusions (both exact):
  scores = (W_q x_q + b_q) . (W_k x_kv + b_k) / sqrt(C)
         = x_q^T M x_kv + c(kv) + g(q),   M = W_q^T W_k / sqrt(C)
    where c_s = b_q . k_s / sqrt(C) enters exp() as a per-kv-partition bias
    (host-computed rank-1 stat: x_kv @ W_k^T b_q + b_q.b_k), and g(q) is
    constant per query column so softmax cancels it -- dropped.
    => the q-projection never runs on device.
  out = softmax(S) V W_p^T + b = (A (W_p W_v) x)/rowsum + (b_p + W_p b_v)
    since softmax rows sum to one => U = W_u x with W_u = W_p W_v replaces
    the V-projection, and the output projection never runs on device.

Device layout per core (S^T formulation -- scores kept as [kv, query] so
att@U needs no transposes):
  zproj: z = M x for all 2048 kv rows into full-width zT[ct][128,2048];
    the first x-half streams against a pair-wise ct-outer matmul order so
    the PE stays ahead of the DMA at kernel start (plus a few 1-row
    const-tile warmup matmuls that start the p-state ramp clock early).
  uproj: U = W_u x into 16 [128,1024] tiles.
  scores: one pass over all 16 kv tiles s against raw x_q on exact column
    ranges [64*s, 1024); exp(score + c_s) on Act -> AT[s] (bf16). The DVE
    accumulates the exp'd tiles into an f32r rs_acc as they appear; two
    f32r ones-matmuls (1 cycle/row) then collapse the 128 kv lanes into
    broadcast row-sums, emitted after the first att@U group so the PE
    never waits on the DVE chain.
  att@U: per output-channel tile, a single PSUM accumulation over all 16
    kv tiles on exact column ranges; the DVE multiplies the fp32 PSUM by
    1/rowsum and adds the folded output bias in-register, and the fp32
    result is the final y^T, DMAd out tile-major.

All matmul operands are bf16 (PSUM accumulates fp32): in the TRN2 cost
model bf16 matmuls run at the same 1 cycle/row as float32r but halve every
DMA transfer and SBUF footprint, letting z/U/x for the full sequence stay
resident with no DRAM spill.
"""

import sys

sys.path.insert(0, "/opt/trn_rl_repo")

import numpy as np

import concourse.bass as bass
import concourse.tile as tile
from concourse import mybir
from concourse.vector_clock import ScopedClock

FP = mybir.dt.float32
FPR = mybir.dt.float32r
BF = mybir.dt.bfloat16
AF = mybir.ActivationFunctionType

P = 128
C = 1024  # embed dim
H = 1024  # query rows per core
T = 2048  # kv length
NT = C // P  # 8 tiles of 128
NKV = T // P  # 16 kv tiles
NEG = -1.0e9

_MAX_WAITS = 1


class _TC(tile.TileContext):
    """TileContext whose tail drain puts its global-clock waits on a nop
    (walrus rejects multi-wait Drain); excess waits are split by
    _split_waits() afterwards."""

    def _drain_and_barrier(self, tick_clock, wait_clock):
        nop_inst = self.nc.sync.nop(nofuse=True, hint="pre_drain_waits")
        wait_clock.add_sem_waits(
            nop_inst.ins, ScopedClock({None: tick_clock.global_clock})
        )
        self.nc.sync.drain()
        self.nc.all_engine_barrier()
        assert self.sems is not None
        popped = self.nc._tile_sem_poison_stack.pop()
        assert popped is self._sem_poison
        self.nc.clear_and_free_semaphores(list(self.sems.allocated().values()))
        self.nc.all_engine_barrier()


def _split_waits(nc, max_waits=_MAX_WAITS):
    """The walrus shipped here rejects instructions carrying more than
    `max_waits` sync waits. Move excess waits onto injected nops placed
    immediately before the instruction on the same engine (identical
    semantics: the engine's sequencer blocks on all of them either way)."""
    import copy

    template = nc.sync.nop(nofuse=True, hint="waitsplit_template").ins
    counter = [0]

    def make_nop(engine, waits):
        nop = copy.deepcopy(template)
        counter[0] += 1
        nop.name = f"I-wsplit-{counter[0]}"
        nop.engine = engine
        nop.sync_info = mybir.SyncInfo(on_wait=list(waits), on_update=[])
        return nop

    f = nc.m.functions[0]
    for bb in f.blocks:
        insts = bb.instructions
        if not any(
            i.sync_info and i.sync_info.on_wait and len(i.sync_info.on_wait) > max_waits
            for i in insts
        ):
            continue
        newlist = []
        for inst in insts:
            si = inst.sync_info
            if si and si.on_wait and len(si.on_wait) > max_waits:
                if inst.name == template.name:
                    newlist.append(inst)
                    continue
                waits = list(si.on_wait)
                del si.on_wait[max_waits:]
                rest = waits[max_waits:]
                while rest:
                    newlist.append(make_nop(inst.engine, rest[:max_waits]))
                    rest = rest[max_waits:]
            newlist.append(inst)
        bb.instructions[:] = newlist


def _chunks(lo, hi, step=512):
    out = []
    while lo < hi:
        w = min(step, hi - lo)
        out.append((lo, lo + w))
        lo += w
    return out


def _build_nc():
    nc = bass.Bass("TRN2", target_bir_lowering=False, debug=False)

    xTq = nc.dram_tensor("xTq", [C, H], BF, kind="ExternalInput").ap()
    xTo = nc.dram_tensor("xTo", [C, H], BF, kind="ExternalInput").ap()
    xTx = nc.dram_tensor("xTx", [C, H], BF, kind="ExternalInput").ap()
    zM = nc.dram_tensor("zM", [C, C], BF, kind="ExternalInput").ap()
    xR = nc.dram_tensor("xR", [T, C], BF, kind="ExternalInput").ap()
    gW = nc.dram_tensor("gW", [C, C], BF, kind="ExternalInput").ap()
    cb = nc.dram_tensor("cb", [P, NKV], FP, kind="ExternalInput").ap()
    beff = nc.dram_tensor("beff", [P, NT], FP, kind="ExternalInput").ap()
    ones_in = nc.dram_tensor("ones_in", [P, P], FPR, kind="ExternalInput").ap()
    beffd = nc.dram_tensor("beffd", [P, C], BF, kind="ExternalInput").ap()
    m64_in = nc.dram_tensor("m64_in", [P, 64], FP, kind="ExternalInput").ap()
    # output in (ot-tile, chunk)-major layout; host reassembles
    yT = nc.dram_tensor("yT", [NT * 2 * P, 512], FP, kind="ExternalOutput").ap()

    with _TC(nc) as tc:
        with (
            tc.tile_pool(name="misc", bufs=1) as misc,
            tc.tile_pool(name="wstream", bufs=3) as wsp,
            tc.tile_pool(name="wcolp", bufs=1) as wcp,
            tc.tile_pool(name="kqv", bufs=1) as kqv,
            tc.tile_pool(name="evac", bufs=5) as evac,
            tc.tile_pool(name="psum", bufs=8, space="PSUM") as pp,
        ):
            m64 = misc.tile([P, 64], FP, tag="m64")
            cb_sb = misc.tile([P, NKV], FP, tag="cb")
            beff_sb = misc.tile([P, NT], FP, tag="beff")
            beffd_sb = misc.tile([P, C], BF, tag="beffd")

            # ---- persistent tensors --------------------------------------
            YT = [kqv.tile([P, H], BF, tag=f"YT{i}", name=f"YT{i}") for i in range(NT)]
            Xr = [kqv.tile([P, C], BF, tag=f"Xr{i}", name=f"Xr{i}") for i in range(NKV)]
            AT = [kqv.tile([P, H], BF, tag=f"AT{i}", name=f"AT{i}") for i in range(NKV)]
            Gn = [kqv.tile([P, H], BF, tag=f"Gn{i}", name=f"Gn{i}") for i in range(NT)]
            rs_acc = kqv.tile([P, H], FPR, tag="rs_acc")
            rs_sb = kqv.tile([P, H], FP, tag="rs_sb")
            ones_r = misc.tile([P, P], FPR, tag="ones_r")

            xho = [kqv.tile([P, H], BF, tag=f"xho{i}", name=f"xho{i}") for i in range(NT)]
            xhx = [kqv.tile([P, H], BF, tag=f"xhx{i}", name=f"xhx{i}") for i in range(NT)]
            xq = [kqv.tile([P, H], BF, tag=f"xq{i}", name=f"xq{i}") for i in range(NT)]
            xhalf = [xho, xhx]

            # =============================================================
            # Projections: z = M x, U = W_u x over all 2048 kv rows
            # =============================================================
            # Warm-up: 1-row matmuls on a framework const tile start the PE
            # p-state ramp clock ~4us before the first real matmul; a few
            # 512-row matmuls on a memset tile then keep the PE from idling
            # long enough (>3us) for the ramp to reset before the first
            # data-dependent matmul issues -- so it runs at full frequency.
            ones1 = nc.const_aps.tensor(1.0, [P, 1], BF)
            dummy_sb = misc.tile([P, 512], BF, tag="dummy")
            nc.gpsimd.memset(dummy_sb[:], 1.0)
            warm_ps = pp.tile([P, 512], FP, tag="ps", name="warm_ps")
            for _ in range(4):
                nc.tensor.matmul(
                    warm_ps[0:1, 0:1], lhsT=ones1, rhs=ones1, start=True, stop=True
                )
            for _ in range(1):
                nc.tensor.matmul(
                    warm_ps[0:1, :], lhsT=ones1, rhs=dummy_sb[:],
                    start=True, stop=True,
                )

            # Interleave the zM-column and x-half-0 loads so both streams
            # arrive just in time for the pair-wise ct-outer start below.
            wz0 = [
                wcp.tile([P, C], BF, tag=f"wz{i}", name=f"wz0_{i}") for i in range(NT)
            ]
            for i, j in ((0, None), (None, 0), (1, None), (None, 1), (None, 2),
                         (2, None), (3, None), (None, 3), (None, 4), (None, 5),
                         (4, None), (5, None), (None, 6), (None, 7),
                         (6, None), (7, None)):
                if i is not None:
                    nc.sync.dma_start(wz0[i][:], zM[i * P : (i + 1) * P, :])
                else:
                    nc.sync.dma_start(xq[j][:], xTq[j * P : (j + 1) * P, :])
            for i in range(NT):
                nc.sync.dma_start(xho[i][:], xTo[i * P : (i + 1) * P, :])

            def yproj(wcols=None, groups=None):
                # z^T: out tile [zc:128, t-chunk], lhsT = M-col slice.
                # `groups` batches ot-tiles with a ct-outer matmul order so
                # each arriving x tile feeds len(group)*1024 rows of PE work
                # (keeps the PE ahead of the x DMA stream at kernel start).
                if groups is None:
                    groups = [[ot] for ot in range(NT)]
                for group in groups:
                    pss = {}
                    if wcols is None:
                        wcols = {}
                    for ot in group:
                        osl = slice(ot * P, (ot + 1) * P)
                        if ot not in wcols:
                            wcols[ot] = wcp.tile(
                                [P, C], BF, tag=f"wz{ot}", name=f"wy_{ot}"
                            )
                            nc.sync.dma_start(wcols[ot][:], zM[osl, :])
                        for (cs, ce) in _chunks(0, H):
                            pss[ot, cs] = pp.tile(
                                [P, 512], FP, tag="ps", name=f"psy_{ot}_{cs}"
                            )
                    for ct in range(NT):
                        for ot in group:
                            for (cs, ce) in _chunks(0, H):
                                nc.tensor.matmul(
                                    pss[ot, cs][:, : ce - cs],
                                    lhsT=wcols[ot][:, ct * P : (ct + 1) * P],
                                    rhs=xq[ct][:, cs:ce],
                                    start=(ct == 0),
                                    stop=(ct == NT - 1),
                                )
                    for ot in group:
                        for (cs, ce) in _chunks(0, H):
                            nc.scalar.activation(
                                YT[ot][:, cs:ce],
                                pss[ot, cs][:, : ce - cs],
                                AF.Identity,
                            )

            sc = tc.nc.named_scope("A_y"); sc.__enter__()
            yproj(wcols=dict(enumerate(wz0)),
                  groups=[[0, 1], [2, 3], [4, 5], [6, 7]])
            sc.__exit__(None, None, None)

            # later loads: queue behind the critical yproj weight stream
            for i in range(NT):
                nc.sync.dma_start(xhx[i][:], xTx[i * P : (i + 1) * P, :])
            nc.sync.dma_start(cb_sb[:], cb[:])
            nc.sync.dma_start(ones_r[:], ones_in[:])
            nc.sync.dma_start(m64[:], m64_in[:])
            nc.sync.dma_start(beff_sb[:], beff[:])
            nc.sync.dma_start(beffd_sb[:], beffd[:])

            for i in range(NKV):
                nc.sync.dma_start(Xr[i][:], xR[i * P : (i + 1) * P, :])

            # =============================================================
            # Attention: scores -> exp -> rowsums, then att@U (one pass)
            # =============================================================
            # kv tile s is valid for local query cols [64*s, 1024): the
            # 64-row query interleave splits each kv tile's diagonal band
            # 50/50 across the core pair, and one s-independent [128,64]
            # mask tile (the core's half of the band) covers the boundary.
            sc = tc.nc.named_scope("S"); sc.__enter__()
            # row-sums: DVE accumulates the exp'd tiles into rs_acc while
            # scores stream; one pair of f32r ones-matmuls then collapses
            # the 128 kv lanes (and broadcasts) -- 1024 PE rows instead of
            # the 8704 a per-tile ones-matmul rowsum would cost.
            for s in range(NKV):
                lo = 64 * s
                for ci, (cs, ce) in enumerate(_chunks(lo, H)):
                    ps = pp.tile([P, 512], FP, tag="ps", name=f"pss{s}_{cs}")
                    w = ce - cs
                    for ct in range(NT):
                        nc.tensor.matmul(
                            ps[:, :w],
                            lhsT=xhalf[s // NT][ct][:, (s % NT) * P : (s % NT + 1) * P],
                            rhs=YT[ct][:, cs:ce],
                            start=(ct == 0),
                            stop=(ct == NT - 1),
                        )
                    if ci == 0:
                        nc.vector.tensor_add(ps[:, 0:64], ps[:, 0:64], m64[:])
                    nc.scalar.activation(
                        AT[s][:, cs:ce], ps[:, :w], AF.Exp,
                        bias=cb_sb[:, s : s + 1],
                    )
                if s == 0:
                    nc.vector.tensor_copy(rs_acc[:], AT[0][:])
                else:
                    nc.vector.tensor_add(
                        rs_acc[:, lo:H], rs_acc[:, lo:H], AT[s][:, lo:H]
                    )
            def rs_collapse():
                for (cs, ce) in _chunks(0, H):
                    ps = pp.tile([P, 512], FP, tag="ps", name=f"psrs_{cs}")
                    nc.tensor.matmul(
                        ps[:], lhsT=ones_r[:], rhs=rs_acc[:, cs:ce],
                        start=True, stop=True,
                    )
                    nc.vector.reciprocal(rs_sb[:, cs:ce], ps[:])
            sc.__exit__(None, None, None)

            sc = tc.nc.named_scope("AX"); sc.__enter__()

            # att@X: G^T[c, q] = sum_s x_kv[s, c-block] AT[s][:, q] -- the
            # raw row-major x tiles are the stationary operand, so the
            # output is the softmax-aggregate of x, to be projected by W_u
            # over this core's 1024 queries only (gproj below)
            def ax_matmuls(ct, cs, ce):
                osl = slice(ct * P, (ct + 1) * P)
                valid = [s for s in range(NKV) if 64 * s < ce]
                ps = pp.tile([P, 512], FP, tag="ps", name=f"psax{ct}_{cs}")
                for s in valid:
                    lo = max(cs, 64 * s)
                    nc.tensor.matmul(
                        ps[:, lo - cs : ce - cs],
                        lhsT=Xr[s][:, osl],
                        rhs=AT[s][:, lo:ce],
                        start=(s == valid[0]),
                        stop=(s == valid[-1]),
                    )
                return ps

            def ax_out(ct, ps, cs, ce):
                # normalize straight out of PSUM into bf16 G tiles
                nc.vector.tensor_mul(
                    Gn[ct][:, cs:ce], ps[:, : ce - cs], rs_sb[:, cs:ce]
                )

            # ct = 0: both matmul groups precede the row-sum lane collapse
            # so the PE's wait on the DVE exp-accumulation chain is hidden
            # under ~3.5us of att@X work
            pss = [ax_matmuls(0, cs, ce) for (cs, ce) in ((0, 512), (512, 1024))]
            rs_collapse()
            for ps, (cs, ce) in zip(pss, ((0, 512), (512, 1024))):
                ax_out(0, ps, cs, ce)
            for ct in range(1, NT):
                for (cs, ce) in ((0, 512), (512, 1024)):
                    ax_out(ct, ax_matmuls(ct, cs, ce), cs, ce)
            sc.__exit__(None, None, None)

            # =============================================================
            # gproj: y^T = W_u G + beff over this core's queries
            # =============================================================
            sc = tc.nc.named_scope("G"); sc.__enter__()
            for ot in range(NT):
                osl = slice(ot * P, (ot + 1) * P)
                wcol = wsp.tile([P, C], BF, tag="wcol", name=f"wg_{ot}")
                nc.sync.dma_start(wcol[:], gW[osl, :])
                if ot < NT - 1:
                    echs = [(0, 512), (512, 1024)]
                else:
                    echs = [(0, 512), (512, 768), (768, 896), (896, 1024)]
                for (cs, ce) in echs:
                    w = ce - cs
                    last = ot == NT - 1 and cs == 896
                    ps = pp.tile([P, 512], FP, tag="ps", name=f"psg{ot}_{cs}")
                    for ct in range(NT):
                        nc.tensor.matmul(
                            ps[:, :w],
                            lhsT=wcol[:, ct * P : (ct + 1) * P],
                            rhs=Gn[ct][:, cs:ce],
                            start=(ct == 0),
                            stop=(ct == NT - 1) and not last,
                        )
                    ev = evac.tile([P, 512], FP, tag="evy")
                    if last:
                        # bias folded into the PSUM via a rank-1 ones-matmul
                        # -> one copy -> DMA: shortest possible tail chain
                        nc.tensor.matmul(
                            ps[:, :w],
                            lhsT=beffd_sb[:, osl],
                            rhs=dummy_sb[:, 0:w],
                            start=False,
                            stop=True,
                        )
                        nc.vector.tensor_copy(ev[:, :w], ps[:, :w])
                    else:
                        nc.scalar.activation(
                            ev[:, :w], ps[:, :w], AF.Identity,
                            bias=beff_sb[:, ot : ot + 1],
                        )
                    ci = cs // 512
                    nc.sync.dma_start(
                        yT[(ot * 2 + ci) * P : (ot * 2 + ci + 1) * P,
                           cs - ci * 512 : ce - ci * 512],
                        ev[:, :w],
                    )
            sc.__exit__(None, None, None)

    _split_waits(nc)
    return nc


_NC_CACHE = None


def _get_nc():
    global _NC_CACHE
    if _NC_CACHE is None:
        _NC_CACHE = _build_nc()
    return _NC_CACHE


def make_in_maps(x, w_qkv, b_qkv, w_proj, b_proj):
    """Host-side prep: weight fusion + shard + transpose + bf16 packing."""
    import ml_dtypes

    BFNP = ml_dtypes.bfloat16
    x = np.asarray(x, dtype=np.float32)
    w_qkv = np.asarray(w_qkv, dtype=np.float32)
    b_qkv = np.asarray(b_qkv, dtype=np.float32)
    w_proj = np.asarray(w_proj, dtype=np.float32)
    b_proj = np.asarray(b_proj, dtype=np.float32)

    s = 1.0 / np.sqrt(np.float32(C))
    Wq = w_qkv[0:C]
    Wk = w_qkv[C : 2 * C]
    Wv = w_qkv[2 * C : 3 * C]
    bqv = b_qkv[0:C]
    bkv = b_qkv[C : 2 * C]
    bvv = b_qkv[2 * C : 3 * C]

    M = (Wq.T @ Wk) * s           # scores main term: x_q^T M x_kv
    Wu = w_proj @ Wv              # fused value/output projection
    beff = b_proj + w_proj @ bvv  # folded output bias
    wc = (Wk.T @ bqv) * s         # c_s = x_s . wc + cconst
    cconst = float(bqv @ bkv) * s

    def pack_cols(w, bw=P):
        # [ot*bw + p(in-part), ct*P + o(out-within)] = w[ot*bw + o, ct*P + p]
        n_o = C // bw
        w4 = w.reshape(n_o, bw, NT, P).transpose(0, 3, 2, 1)
        return np.ascontiguousarray(w4).reshape(n_o * P, NT * bw).astype(BFNP)

    zM = pack_cols(np.ascontiguousarray(M.T))
    gWp = pack_cols(Wu)
    beff_t = np.ascontiguousarray(beff.reshape(NT, P).T)

    # S^T layout: partition = kv index j (0..127 within a kv tile), free =
    # the first valid 64 local query cols; the core sees global query rows
    # 64*h + i2 of the tile's diagonal band: visible iff 64*h + i2 >= j
    jj = np.arange(P)[:, None]
    ii = np.arange(64)[None, :]
    shared = dict(
        zM=zM, gW=gWp, beff=beff_t,
        ones_in=np.ones((P, P), dtype=np.float32),
        # bias / 128 broadcast down the contraction partitions: a rank-1
        # ones-matmul reconstitutes beff inside the final output PSUM
        beffd=np.ascontiguousarray(
            np.broadcast_to((beff / P)[None, :], (P, C))
        ).astype(BFNP),
    )
    in_maps = []
    for core in range(8):
        b, h = core // 2, core % 2
        m64 = np.where(64 * h + ii >= jj, 0.0, NEG).astype(np.float32)
        xb = x[b]  # [T, C]
        # per-kv-row score bias c_s, laid out [128, 16] kv-tile-major
        c = (xb @ wc + cconst).astype(np.float32)  # [T]
        cb = np.ascontiguousarray(c.reshape(NKV, P).T)
        # query rows: interleaved 64-blocks g = 2*i + h
        qrows = np.concatenate(
            [xb[(2 * i + h) * 64 : (2 * i + h + 1) * 64] for i in range(H // 64)],
            axis=0,
        )
        xTo_b = np.ascontiguousarray(xb[0:H].T).astype(BFNP)
        xR_b = np.ascontiguousarray(xb).astype(BFNP)
        in_maps.append(
            dict(
                shared,
                xTq=np.ascontiguousarray(qrows.T).astype(BFNP),
                xTo=xTo_b,
                xTx=np.ascontiguousarray(xb[H : 2 * H].T).astype(BFNP),
                xR=xR_b,
                cb=cb,
                m64_in=m64,
            )
        )
    return in_maps


def assemble_output(results):
    B = 4
    y = np.empty((B, T, C), dtype=np.float32)
    for core in range(8):
        b, h = core // 2, core % 2
        # yT layout [ot, ci, p, 512] -> rows are local query cols
        yt = results[core]["yT"].reshape(NT, 2, P, 512)
        blk = yt.transpose(1, 3, 0, 2).reshape(H, C)  # [local q, C]
        blk16 = blk.reshape(H // 64, 64, C)
        for i in range(H // 64):
            g = 2 * i + h
            y[b, g * 64 : (g + 1) * 64, :] = blk16[i]
    return y


def kernel(x, w_qkv, b_qkv, w_proj, b_proj):
    from concourse.bass_utils import run_bass_kernel_spmd

    nc = _get_nc()
    in_maps = make_in_maps(x, w_qkv, b_qkv, w_proj, b_proj)
    res = run_bass_kernel_spmd(nc, in_maps, list(range(8)))
    return assemble_output(res.results)


# revision 95
# speedup vs baseline: 1.4454x; 1.0023x over previous
"""Single-head causal attention (B=4, T=2048, C=1024) on 8 trn2 NeuronCores.

Sharding: 8 shards = (batch b in 0..3) x (query interleave h in 0..1).
Query rows are sharded as interleaved 64-row blocks (core h takes global
blocks {2*i+h}), which splits every kv tile's 128-row causal diagonal band
exactly 50/50 across the core pair: kv tile s is valid for local query
cols [64*s, 1024) on BOTH cores. One SPMD instruction stream serves all
cores; all per-core variation is data: gathered x slices, a per-kv-row
score bias, and a single [128,64] mask tile holding this core's half of
the diagonal band.

Host-side weight fusions (both exact):
  scores = (W_q x_q + b_q) . (W_k x_kv + b_k) / sqrt(C)
         = x_q^T M x_kv + c(kv) + g(q),   M = W_q^T W_k / sqrt(C)
    where c_s = b_q . k_s / sqrt(C) enters exp() as a per-kv-partition bias
    (host-computed rank-1 stat: x_kv @ W_k^T b_q + b_q.b_k), and g(q) is
    constant per query column so softmax cancels it -- dropped.
    => the q-projection never runs on device.
  out = softmax(S) V W_p^T + b = (A (W_p W_v) x)/rowsum + (b_p + W_p b_v)
    since softmax rows sum to one => U = W_u x with W_u = W_p W_v replaces
    the V-projection, and the output projection never runs on device.

Device layout per core (S^T formulation -- scores kept as [kv, query] so
att@U needs no transposes):
  zproj: z = M x for all 2048 kv rows into full-width zT[ct][128,2048];
    the first x-half streams against a pair-wise ct-outer matmul order so
    the PE stays ahead of the DMA at kernel start (plus a few 1-row
    const-tile warmup matmuls that start the p-state ramp clock early).
  uproj: U = W_u x into 16 [128,1024] tiles.
  scores: one pass over all 16 kv tiles s against raw x_q on exact column
    ranges [64*s, 1024); exp(score + c_s) on Act -> AT[s] (bf16). The DVE
    accumulates the exp'd tiles into an f32r rs_acc as they appear; two
    f32r ones-matmuls (1 cycle/row) then collapse the 128 kv lanes into
    broadcast row-sums, emitted after the first att@U group so the PE
    never waits on the DVE chain.
  att@U: per output-channel tile, a single PSUM accumulation over all 16
    kv tiles on exact column ranges; the DVE multiplies the fp32 PSUM by
    1/rowsum and adds the folded output bias in-register, and the fp32
    result is the final y^T, DMAd out tile-major.

All matmul operands are bf16 (PSUM accumulates fp32): in the TRN2 cost
model bf16 matmuls run at the same 1 cycle/row as float32r but halve every
DMA transfer and SBUF footprint, letting z/U/x for the full sequence stay
resident with no DRAM spill.
"""

import sys

sys.path.insert(0, "/opt/trn_rl_repo")

import numpy as np

import concourse.bass as bass
import concourse.tile as tile
from concourse import mybir
from concourse.vector_clock import ScopedClock

FP = mybir.dt.float32
FPR = mybir.dt.float32r
BF = mybir.dt.bfloat16
AF = mybir.ActivationFunctionType

P = 128
C = 1024  # embed dim
H = 1024  # query rows per core
T = 2048  # kv length
NT = C // P  # 8 tiles of 128
NKV = T // P  # 16 kv tiles
NEG = -1.0e9

_MAX_WAITS = 1


class _TC(tile.TileContext):
    """TileContext whose tail drain puts its global-clock waits on a nop
    (walrus rejects multi-wait Drain); excess waits are split by
    _split_waits() afterwards."""

    def _drain_and_barrier(self, tick_clock, wait_clock):
        nop_inst = self.nc.sync.nop(nofuse=True, hint="pre_drain_waits")
        wait_clock.add_sem_waits(
            nop_inst.ins, ScopedClock({None: tick_clock.global_clock})
        )
        self.nc.sync.drain()
        self.nc.all_engine_barrier()
        assert self.sems is not None
        popped = self.nc._tile_sem_poison_stack.pop()
        assert popped is self._sem_poison
        self.nc.clear_and_free_semaphores(list(self.sems.allocated().values()))
        self.nc.all_engine_barrier()


def _split_waits(nc, max_waits=_MAX_WAITS):
    """The walrus shipped here rejects instructions carrying more than
    `max_waits` sync waits. Move excess waits onto injected nops placed
    immediately before the instruction on the same engine (identical
    semantics: the engine's sequencer blocks on all of them either way)."""
    import copy

    template = nc.sync.nop(nofuse=True, hint="waitsplit_template").ins
    counter = [0]

    def make_nop(engine, waits):
        nop = copy.deepcopy(template)
        counter[0] += 1
        nop.name = f"I-wsplit-{counter[0]}"
        nop.engine = engine
        nop.sync_info = mybir.SyncInfo(on_wait=list(waits), on_update=[])
        return nop

    f = nc.m.functions[0]
    for bb in f.blocks:
        insts = bb.instructions
        if not any(
            i.sync_info and i.sync_info.on_wait and len(i.sync_info.on_wait) > max_waits
            for i in insts
        ):
            continue
        newlist = []
        for inst in insts:
            si = inst.sync_info
            if si and si.on_wait and len(si.on_wait) > max_waits:
                if inst.name == template.name:
                    newlist.append(inst)
                    continue
                waits = list(si.on_wait)
                del si.on_wait[max_waits:]
                rest = waits[max_waits:]
                while rest:
                    newlist.append(make_nop(inst.engine, rest[:max_waits]))
                    rest = rest[max_waits:]
            newlist.append(inst)
        bb.instructions[:] = newlist


def _chunks(lo, hi, step=512):
    out = []
    while lo < hi:
        w = min(step, hi - lo)
        out.append((lo, lo + w))
        lo += w
    return out


def _build_nc():
    nc = bass.Bass("TRN2", target_bir_lowering=False, debug=False)

    xTq = nc.dram_tensor("xTq", [C, H], BF, kind="ExternalInput").ap()
    xTo = nc.dram_tensor("xTo", [C, H], BF, kind="ExternalInput").ap()
    xTx = nc.dram_tensor("xTx", [C, H], BF, kind="ExternalInput").ap()
    zM = nc.dram_tensor("zM", [C, C], BF, kind="ExternalInput").ap()
    xR = nc.dram_tensor("xR", [T, C], BF, kind="ExternalInput").ap()
    gW = nc.dram_tensor("gW", [C, C], BF, kind="ExternalInput").ap()
    cb = nc.dram_tensor("cb", [P, NKV], FP, kind="ExternalInput").ap()
    beff = nc.dram_tensor("beff", [P, NT], FP, kind="ExternalInput").ap()
    ones_in = nc.dram_tensor("ones_in", [P, P], FPR, kind="ExternalInput").ap()
    beffd = nc.dram_tensor("beffd", [P, C], BF, kind="ExternalInput").ap()
    m64_in = nc.dram_tensor("m64_in", [P, 64], FP, kind="ExternalInput").ap()
    # output in (ot-tile, chunk)-major layout; host reassembles
    yT = nc.dram_tensor("yT", [NT * 2 * P, 512], FP, kind="ExternalOutput").ap()

    with _TC(nc) as tc:
        with (
            tc.tile_pool(name="misc", bufs=1) as misc,
            tc.tile_pool(name="wstream", bufs=3) as wsp,
            tc.tile_pool(name="wcolp", bufs=1) as wcp,
            tc.tile_pool(name="kqv", bufs=1) as kqv,
            tc.tile_pool(name="evac", bufs=5) as evac,
            tc.tile_pool(name="psum", bufs=8, space="PSUM") as pp,
        ):
            m64 = misc.tile([P, 64], FP, tag="m64")
            cb_sb = misc.tile([P, NKV], FP, tag="cb")
            beff_sb = misc.tile([P, NT], FP, tag="beff")
            beffd_sb = misc.tile([P, C], BF, tag="beffd")

            # ---- persistent tensors --------------------------------------
            YT = [kqv.tile([P, H], BF, tag=f"YT{i}", name=f"YT{i}") for i in range(NT)]
            Xr = [kqv.tile([P, C], BF, tag=f"Xr{i}", name=f"Xr{i}") for i in range(NKV)]
            AT = [kqv.tile([P, H], BF, tag=f"AT{i}", name=f"AT{i}") for i in range(NKV)]
            Gn = [kqv.tile([P, H], BF, tag=f"Gn{i}", name=f"Gn{i}") for i in range(NT)]
            rs_acc = kqv.tile([P, H], FPR, tag="rs_acc")
            rs_sb = kqv.tile([P, H], FP, tag="rs_sb")
            ones_r = misc.tile([P, P], FPR, tag="ones_r")

            xho = [kqv.tile([P, H], BF, tag=f"xho{i}", name=f"xho{i}") for i in range(NT)]
            xhx = [kqv.tile([P, H], BF, tag=f"xhx{i}", name=f"xhx{i}") for i in range(NT)]
            xq = [kqv.tile([P, H], BF, tag=f"xq{i}", name=f"xq{i}") for i in range(NT)]
            xhalf = [xho, xhx]

            # =============================================================
            # Projections: z = M x, U = W_u x over all 2048 kv rows
            # =============================================================
            # Warm-up: 1-row matmuls on a framework const tile start the PE
            # p-state ramp clock ~4us before the first real matmul; a few
            # 512-row matmuls on a memset tile then keep the PE from idling
            # long enough (>3us) for the ramp to reset before the first
            # data-dependent matmul issues -- so it runs at full frequency.
            ones1 = nc.const_aps.tensor(1.0, [P, 1], BF)
            dummy_sb = misc.tile([P, 512], BF, tag="dummy")
            nc.gpsimd.memset(dummy_sb[:], 1.0)
            warm_ps = pp.tile([P, 512], FP, tag="ps", name="warm_ps")
            for _ in range(4):
                nc.tensor.matmul(
                    warm_ps[0:1, 0:1], lhsT=ones1, rhs=ones1, start=True, stop=True
                )
            for _ in range(1):
                nc.tensor.matmul(
                    warm_ps[0:1, :], lhsT=ones1, rhs=dummy_sb[:],
                    start=True, stop=True,
                )

            # Interleave the zM-column and x-half-0 loads so both streams
            # arrive just in time for the pair-wise ct-outer start below.
            wz0 = [
                wcp.tile([P, C], BF, tag=f"wz{i}", name=f"wz0_{i}") for i in range(NT)
            ]
            for i, j in ((0, None), (None, 0), (1, None), (None, 1), (None, 2),
                         (2, None), (3, None), (None, 3), (None, 4), (None, 5),
                         (4, None), (5, None), (None, 6), (None, 7),
                         (6, None), (7, None)):
                if i is not None:
                    nc.sync.dma_start(wz0[i][:], zM[i * P : (i + 1) * P, :])
                else:
                    nc.sync.dma_start(xq[j][:], xTq[j * P : (j + 1) * P, :])
            for i in range(NT):
                nc.sync.dma_start(xho[i][:], xTo[i * P : (i + 1) * P, :])

            def yproj(wcols=None, groups=None):
                # z^T: out tile [zc:128, t-chunk], lhsT = M-col slice.
                # `groups` batches ot-tiles with a ct-outer matmul order so
                # each arriving x tile feeds len(group)*1024 rows of PE work
                # (keeps the PE ahead of the x DMA stream at kernel start).
                if groups is None:
                    groups = [[ot] for ot in range(NT)]
                for group in groups:
                    pss = {}
                    if wcols is None:
                        wcols = {}
                    for ot in group:
                        osl = slice(ot * P, (ot + 1) * P)
                        if ot not in wcols:
                            wcols[ot] = wcp.tile(
                                [P, C], BF, tag=f"wz{ot}", name=f"wy_{ot}"
                            )
                            nc.sync.dma_start(wcols[ot][:], zM[osl, :])
                        for (cs, ce) in _chunks(0, H):
                            pss[ot, cs] = pp.tile(
                                [P, 512], FP, tag="ps", name=f"psy_{ot}_{cs}"
                            )
                    for ct in range(NT):
                        for ot in group:
                            for (cs, ce) in _chunks(0, H):
                                nc.tensor.matmul(
                                    pss[ot, cs][:, : ce - cs],
                                    lhsT=wcols[ot][:, ct * P : (ct + 1) * P],
                                    rhs=xq[ct][:, cs:ce],
                                    start=(ct == 0),
                                    stop=(ct == NT - 1),
                                )
                    for ot in group:
                        for (cs, ce) in _chunks(0, H):
                            nc.scalar.activation(
                                YT[ot][:, cs:ce],
                                pss[ot, cs][:, : ce - cs],
                                AF.Identity,
                            )

            sc = tc.nc.named_scope("A_y"); sc.__enter__()
            yproj(wcols=dict(enumerate(wz0)),
                  groups=[[0, 1], [2, 3], [4, 5], [6, 7]])
            sc.__exit__(None, None, None)

            # later loads: queue behind the critical yproj weight stream
            for i in range(NT):
                nc.sync.dma_start(xhx[i][:], xTx[i * P : (i + 1) * P, :])
            nc.sync.dma_start(cb_sb[:], cb[:])
            nc.sync.dma_start(ones_r[:], ones_in[:])
            nc.sync.dma_start(m64[:], m64_in[:])
            nc.sync.dma_start(beff_sb[:], beff[:])
            nc.sync.dma_start(beffd_sb[:], beffd[:])

            for i in range(NKV):
                nc.sync.dma_start(Xr[i][:], xR[i * P : (i + 1) * P, :])

            # =============================================================
            # Attention: scores -> exp -> rowsums, then att@U (one pass)
            # =============================================================
            # kv tile s is valid for local query cols [64*s, 1024): the
            # 64-row query interleave splits each kv tile's diagonal band
            # 50/50 across the core pair, and one s-independent [128,64]
            # mask tile (the core's half of the band) covers the boundary.
            sc = tc.nc.named_scope("S"); sc.__enter__()
            # row-sums: DVE accumulates the exp'd tiles into rs_acc while
            # scores stream; one pair of f32r ones-matmuls then collapses
            # the 128 kv lanes (and broadcasts) -- 1024 PE rows instead of
            # the 8704 a per-tile ones-matmul rowsum would cost.
            for s in range(NKV):
                lo = 64 * s
                for ci, (cs, ce) in enumerate(_chunks(lo, H)):
                    ps = pp.tile([P, 512], FP, tag="ps", name=f"pss{s}_{cs}")
                    w = ce - cs
                    for ct in range(NT):
                        nc.tensor.matmul(
                            ps[:, :w],
                            lhsT=xhalf[s // NT][ct][:, (s % NT) * P : (s % NT + 1) * P],
                            rhs=YT[ct][:, cs:ce],
                            start=(ct == 0),
                            stop=(ct == NT - 1),
                        )
                    if ci == 0:
                        nc.vector.tensor_add(ps[:, 0:64], ps[:, 0:64], m64[:])
                    nc.scalar.activation(
                        AT[s][:, cs:ce], ps[:, :w], AF.Exp,
                        bias=cb_sb[:, s : s + 1],
                    )
                if s == 0:
                    nc.vector.tensor_copy(rs_acc[:], AT[0][:])
                else:
                    nc.vector.tensor_add(
                        rs_acc[:, lo:H], rs_acc[:, lo:H], AT[s][:, lo:H]
                    )
            def rs_collapse():
                for (cs, ce) in _chunks(0, H):
                    ps = pp.tile([P, 512], FP, tag="ps", name=f"psrs_{cs}")
                    nc.tensor.matmul(
                        ps[:], lhsT=ones_r[:], rhs=rs_acc[:, cs:ce],
                        start=True, stop=True,
                    )
                    nc.vector.reciprocal(rs_sb[:, cs:ce], ps[:])
            sc.__exit__(None, None, None)

            sc = tc.nc.named_scope("AX"); sc.__enter__()

            # att@X: G^T[c, q] = sum_s x_kv[s, c-block] AT[s][:, q] -- the
            # raw row-major x tiles are the stationary operand, so the
            # output is the softmax-aggregate of x, to be projected by W_u
            # over this core's 1024 queries only (gproj below)
            def ax_matmuls(ct, cs, ce):
                osl = slice(ct * P, (ct + 1) * P)
                valid = [s for s in range(NKV) if 64 * s < ce]
                ps = pp.tile([P, 512], FP, tag="ps", name=f"psax{ct}_{cs}")
                for s in valid:
                    lo = max(cs, 64 * s)
                    nc.tensor.matmul(
                        ps[:, lo - cs : ce - cs],
                        lhsT=Xr[s][:, osl],
                        rhs=AT[s][:, lo:ce],
                        start=(s == valid[0]),
                        stop=(s == valid[-1]),
                    )
                return ps

            def ax_out(ct, ps, cs, ce):
                # normalize straight out of PSUM into bf16 G tiles
                nc.vector.tensor_mul(
                    Gn[ct][:, cs:ce], ps[:, : ce - cs], rs_sb[:, cs:ce]
                )

            # ct = 0: both matmul groups precede the row-sum lane collapse
            # so the PE's wait on the DVE exp-accumulation chain is hidden
            # under ~3.5us of att@X work
            pss = [ax_matmuls(0, cs, ce) for (cs, ce) in ((0, 512), (512, 1024))]
            rs_collapse()
            for ps, (cs, ce) in zip(pss, ((0, 512), (512, 1024))):
                ax_out(0, ps, cs, ce)
            for ct in range(1, NT):
                for (cs, ce) in ((0, 512), (512, 1024)):
                    ax_out(ct, ax_matmuls(ct, cs, ce), cs, ce)
            sc.__exit__(None, None, None)

            # =============================================================
            # gproj: y^T = W_u G + beff over this core's queries
            # =============================================================
            sc = tc.nc.named_scope("G"); sc.__enter__()
            for ot in range(NT):
                osl = slice(ot * P, (ot + 1) * P)
                wcol = wsp.tile([P, C], BF, tag="wcol", name=f"wg_{ot}")
                nc.sync.dma_start(wcol[:], gW[osl, :])
                if ot < NT - 1:
                    echs = [(0, 512), (512, 1024)]
                else:
                    # final chunk last; the other chunks' DMAs issue first
                    # so their HWDGE slots clear before its critical chain
                    echs = [(768, 896), (0, 512), (512, 768), (896, 1024)]
                for (cs, ce) in echs:
                    w = ce - cs
                    last = ot == NT - 1 and cs == 896
                    ps = pp.tile([P, 512], FP, tag="ps", name=f"psg{ot}_{cs}")
                    for ct in range(NT):
                        nc.tensor.matmul(
                            ps[:, :w],
                            lhsT=wcol[:, ct * P : (ct + 1) * P],
                            rhs=Gn[ct][:, cs:ce],
                            start=(ct == 0),
                            stop=(ct == NT - 1) and not last,
                        )
                    ev = evac.tile([P, 512], FP, tag="evy")
                    if last:
                        # bias folded into the PSUM via a rank-1 ones-matmul
                        # -> one copy -> DMA: shortest possible tail chain
                        nc.tensor.matmul(
                            ps[:, :w],
                            lhsT=beffd_sb[:, osl],
                            rhs=dummy_sb[:, 0:w],
                            start=False,
                            stop=True,
                        )
                        nc.vector.tensor_copy(ev[:, :w], ps[:, :w])
                    else:
                        nc.scalar.activation(
                            ev[:, :w], ps[:, :w], AF.Identity,
                            bias=beff_sb[:, ot : ot + 1],
                        )
                    ci = cs // 512
                    nc.sync.dma_start(
                        yT[(ot * 2 + ci) * P : (ot * 2 + ci + 1) * P,
                           cs - ci * 512 : ce - ci * 512],
                        ev[:, :w],
                    )
            sc.__exit__(None, None, None)

    _split_waits(nc)
    return nc


_NC_CACHE = None


def _get_nc():
    global _NC_CACHE
    if _NC_CACHE is None:
        _NC_CACHE = _build_nc()
    return _NC_CACHE


def make_in_maps(x, w_qkv, b_qkv, w_proj, b_proj):
    """Host-side prep: weight fusion + shard + transpose + bf16 packing."""
    import ml_dtypes

    BFNP = ml_dtypes.bfloat16
    x = np.asarray(x, dtype=np.float32)
    w_qkv = np.asarray(w_qkv, dtype=np.float32)
    b_qkv = np.asarray(b_qkv, dtype=np.float32)
    w_proj = np.asarray(w_proj, dtype=np.float32)
    b_proj = np.asarray(b_proj, dtype=np.float32)

    s = 1.0 / np.sqrt(np.float32(C))
    Wq = w_qkv[0:C]
    Wk = w_qkv[C : 2 * C]
    Wv = w_qkv[2 * C : 3 * C]
    bqv = b_qkv[0:C]
    bkv = b_qkv[C : 2 * C]
    bvv = b_qkv[2 * C : 3 * C]

    M = (Wq.T @ Wk) * s           # scores main term: x_q^T M x_kv
    Wu = w_proj @ Wv              # fused value/output projection
    beff = b_proj + w_proj @ bvv  # folded output bias
    wc = (Wk.T @ bqv) * s         # c_s = x_s . wc + cconst
    cconst = float(bqv @ bkv) * s

    def pack_cols(w, bw=P):
        # [ot*bw + p(in-part), ct*P + o(out-within)] = w[ot*bw + o, ct*P + p]
        n_o = C // bw
        w4 = w.reshape(n_o, bw, NT, P).transpose(0, 3, 2, 1)
        return np.ascontiguousarray(w4).reshape(n_o * P, NT * bw).astype(BFNP)

    zM = pack_cols(np.ascontiguousarray(M.T))
    gWp = pack_cols(Wu)
    beff_t = np.ascontiguousarray(beff.reshape(NT, P).T)

    # S^T layout: partition = kv index j (0..127 within a kv tile), free =
    # the first valid 64 local query cols; the core sees global query rows
    # 64*h + i2 of the tile's diagonal band: visible iff 64*h + i2 >= j
    jj = np.arange(P)[:, None]
    ii = np.arange(64)[None, :]
    shared = dict(
        zM=zM, gW=gWp, beff=beff_t,
        ones_in=np.ones((P, P), dtype=np.float32),
        # bias / 128 broadcast down the contraction partitions: a rank-1
        # ones-matmul reconstitutes beff inside the final output PSUM
        beffd=np.ascontiguousarray(
            np.broadcast_to((beff / P)[None, :], (P, C))
        ).astype(BFNP),
    )
    in_maps = []
    for core in range(8):
        b, h = core // 2, core % 2
        m64 = np.where(64 * h + ii >= jj, 0.0, NEG).astype(np.float32)
        xb = x[b]  # [T, C]
        # per-kv-row score bias c_s, laid out [128, 16] kv-tile-major
        c = (xb @ wc + cconst).astype(np.float32)  # [T]
        cb = np.ascontiguousarray(c.reshape(NKV, P).T)
        # query rows: interleaved 64-blocks g = 2*i + h
        qrows = np.concatenate(
            [xb[(2 * i + h) * 64 : (2 * i + h + 1) * 64] for i in range(H // 64)],
            axis=0,
        )
        xTo_b = np.ascontiguousarray(xb[0:H].T).astype(BFNP)
        xR_b = np.ascontiguousarray(xb).astype(BFNP)
        in_maps.append(
            dict(
                shared,
                xTq=np.ascontiguousarray(qrows.T).astype(BFNP),
                xTo=xTo_b,
                xTx=np.ascontiguousarray(xb[H : 2 * H].T).astype(BFNP),
                xR=xR_b,
                cb=cb,
                m64_in=m64,
            )
        )
    return in_maps


def assemble_output(results):
    B = 4
    y = np.empty((B, T, C), dtype=np.float32)
    for core in range(8):
        b, h = core // 2, core % 2
        # yT layout [ot, ci, p, 512] -> rows are local query cols
        yt = results[core]["yT"].reshape(NT, 2, P, 512)
        blk = yt.transpose(1, 3, 0, 2).reshape(H, C)  # [local q, C]
        blk16 = blk.reshape(H // 64, 64, C)
        for i in range(H // 64):
            g = 2 * i + h
            y[b, g * 64 : (g + 1) * 64, :] = blk16[i]
    return y


def kernel(x, w_qkv, b_qkv, w_proj, b_proj):
    from concourse.bass_utils import run_bass_kernel_spmd

    nc = _get_nc()
    in_maps = make_in_maps(x, w_qkv, b_qkv, w_proj, b_proj)
    res = run_bass_kernel_spmd(nc, in_maps, list(range(8)))
    return assemble_output(res.results)
